# revision 15
# baseline (speedup 1.0000x reference)
"""Trainium2 Bass kernel for nn_AttnAggregator (GAT-style neighbor aggregation).

Reference computation:
    ep = embed_table @ W_proj.T                      # [N, 64]
    neigh = ep[padded_neighs]                        # [B, L, 64]
    scores = leaky_relu(ep[dst]@a_dst) + leaky_relu(neigh@a_src)
    attn = masked_softmax(scores, axis=L)
    out = sum_l attn * neigh                         # [B, 64]

Algebraic facts used:
  * The dst term is constant along the softmax axis L, so it cancels — the
    output does not depend on dst_idx / a_dst.
  * Masked neighbors have exactly zero softmax weight (exp(-1e9) underflows),
    so only unmasked edges are processed.
  * es(n) = exp(leaky_relu(ep[n]@a_src)) is a pure per-table-row quantity, and
    score(n) = ep[n]@a_src = embed[n]@(W_proj.T@a_src) = embed[n]@v.  So each
    table row can be pre-weighted: row(n) = [es(n)*ep(n,:), es(n)] and
        out[b] = (sum_l row(idx[b,l])[0:64]) / (sum_l row(idx[b,l])[64]).

Strategy (8 NeuronCores):
  Launch 1 (projection, table-row-sharded): each core projects N/8 = 25000
    rows in fp16: one PE matmul per 128-row chunk computes [ep | s] with a
    fused rhs [W^T | v], ACT exp(leaky) + per-partition scale produce fp16
    rows [es*ep | es | pad] written with 256B stride to HBM.
  Launch 2 (aggregation, batch-sharded): each core aggregates B/8 = 6250
    nodes.  Edges are bucketed host-side by (128-node window, 32768-row table
    shard); buckets are concatenated per shard and gathered by
    InstDMAGatherAnt ("dma_gather") SWDGE instructions with int16 shard-local
    indices, 1024 rows per instruction, rotated across SWDGE queues 0-3 so
    all four Q7 pairs generate descriptors concurrently (~3x over one queue).
    Gathered rows land packed [128, chunk, 65] fp16; host-built fp8 one-hot
    matrices map chunk positions to window nodes, so one PE matmul per
    (bucket, chunk) overlap (onehot^T @ rows, PSUM-accumulated per bucket)
    performs the entire segment-sum; bucket partials are DVE-added into
    per-window SBUF accumulators [sum es*ep | sum es], and a final DVE pass
    divides and streams out.  Each bucket's matmuls are consecutive PE
    instructions — interleaved PSUM accumulation groups are broken on HW.
  (dma_gather elem_size of 65 fp16 = 130B is built via a raw InstDMAGatherAnt
   — the bass-level %256 assert is a transpose-mode-only restriction;
   verified on hardware.)
"""

import os
import sys

sys.path.insert(0, "/opt/trn_rl_repo")

import numpy as np

# ---- hardcoded problem shapes -------------------------------------------------
B, L, N = 50000, 50, 200000
D_IN, D_OUT = 256, 64
NCORES = 8
R = N // NCORES        # 25000 table rows per core (launch 1)
BN = B // NCORES       # 6250 batch nodes per core (launch 2)
P = 128
ROWE = 128             # fp16 elements per padded table row (256B stride)
PAY = D_OUT + 1        # 65 gathered elements: [64 weighted feats | es]
SHARD = 32768          # dma_gather int16 index range per table slice
NSHARDS = (N + SHARD - 1) // SHARD   # 7
WND = (BN + P - 1) // P              # 49 node windows per core
SUBCH = 8              # 128-row chunks per dma_gather (1024 idx; 2048 hangs)
OHBLK = 64             # one-hot matrices per DMA block

_CACHE = {}
LAST_PERF = []         # filled when KERNEL_TRACE=1: list of BassKernelResults
LAST_TW = None         # debug: the fp16 weighted table from launch 1


def _raw_dma_gather(g, out_ap, in_ap, idxs_ap, num_idxs, elem_size, elem_step,
                    queue_num=0):
    """dma_gather with arbitrary elem_size (the bass %256 assert is a
    transpose-mode restriction; non-transpose handles any element bytes)."""
    from concourse import mybir
    from concourse._compat import exact_div

    stride_bytes = elem_step * mybir.dt.size(in_ap.dtype)
    return g.add_instruction(
        mybir.InstDMAGatherAnt(
            name=g.bass.get_next_instruction_name(),
            ins=[
                *g.lower_ap_dma(in_ap, for_custom_bir_dma=True),
                g.lower_ap(idxs_ap),
                g.lower_val_access(g.to_reg(num_idxs)),
            ],
            outs=[g.lower_ap(out_ap)],
            transpose=False,
            num_idxs=num_idxs,
            elem_size=elem_size,
            stride_bytes_256=exact_div(stride_bytes, 256),
            gen_mode=0,
            single_packet=True,
            queue_num=queue_num,
            sbuf_tokens_per_rank=0,
            sbuf_free_dim_per_rank=0,
            sbuf_free_dim_pad_per_rank=0,
            sbuf_byte_offset=0,
        )
    )


def _build_proj():
    """Launch 1: tw[r, :] = [es*ep (64) | es | garbage pad] fp16, 256B stride."""
    from concourse import bacc, mybir
    from concourse.tile import TileContext
    from contextlib import ExitStack

    F32 = mybir.dt.float32
    F16 = mybir.dt.float16
    nc = bacc.Bacc("TRN2", target_bir_lowering=False)
    tT = nc.dram_tensor("tT", [D_IN, R], F16, kind="ExternalInput")
    wv = nc.dram_tensor("wv", [P, 2, PAY], F16, kind="ExternalInput")
    tw = nc.dram_tensor("tw", [R, ROWE], F16, kind="ExternalOutput")

    CPB = 7
    BLK = P * CPB

    with TileContext(nc) as tc, ExitStack() as ctx:
        singles = ctx.enter_context(tc.tile_pool(name="singles", bufs=1))
        tpool = ctx.enter_context(tc.tile_pool(name="tpool", bufs=4))
        spool = ctx.enter_context(tc.tile_pool(name="spool", bufs=6))
        rpool = ctx.enter_context(tc.tile_pool(name="rpool", bufs=4))
        psum = ctx.enter_context(tc.tile_pool(name="psum", bufs=6, space="PSUM"))

        wv_sb = singles.tile([P, 2, PAY], F16)
        nc.sync.dma_start(out=wv_sb[:], in_=wv[:, :, :])
        tTr = tT.rearrange("(k p) r -> p k r", p=P)

        for B0 in range(0, R, BLK):
            wcols = min(BLK, R - B0)
            nj = (wcols + P - 1) // P
            tt = tpool.tile([P, 2, BLK], F16)
            nc.sync.dma_start(out=tt[:, :, :wcols], in_=tTr[:, :, B0 : B0 + wcols])
            pse = psum.tile([P, CPB, PAY], F32, space="PSUM")
            for j in range(nj):
                c0 = B0 + j * P
                cw = min(P, R - c0)
                nc.tensor.matmul(
                    pse[:cw, j, :], tt[:, 0, j * P : j * P + cw], wv_sb[:, 0, :],
                    start=True, stop=False,
                )
                nc.tensor.matmul(
                    pse[:cw, j, :], tt[:, 1, j * P : j * P + cw], wv_sb[:, 1, :],
                    start=False, stop=True,
                )
            sc = spool.tile([P, CPB], F32)
            nc.vector.tensor_copy(out=sc[:, :nj], in_=pse[:, :nj, D_OUT])
            lr = spool.tile([P, CPB], F32)
            nc.vector.scalar_tensor_tensor(
                out=lr[:, :nj],
                in0=sc[:, :nj],
                scalar=0.2,
                in1=sc[:, :nj],
                op0=mybir.AluOpType.mult,
                op1=mybir.AluOpType.max,
            )
            es = spool.tile([P, CPB], F32)
            nc.scalar.activation(
                out=es[:, :nj], in_=lr[:, :nj],
                func=mybir.ActivationFunctionType.Exp,
            )
            RT = rpool.tile([P, CPB, ROWE], F16)
            esb = es[:, :nj].to_broadcast([P, nj, D_OUT])
            nc.vector.tensor_tensor(
                out=RT[:, :nj, 0:D_OUT], in0=pse[:, :nj, 0:D_OUT], in1=esb,
                op=mybir.AluOpType.mult,
            )
            nc.vector.tensor_copy(out=RT[:, :nj, D_OUT], in_=es[:, :nj])
            if wcols == BLK:
                nc.sync.dma_start(
                    out=tw[B0 : B0 + BLK, :].rearrange("(j p) e -> p j e", p=P),
                    in_=RT[:, :, :],
                )
            else:
                for j in range(nj):
                    c0 = B0 + j * P
                    cw = min(P, R - c0)
                    nc.sync.dma_start(out=tw[c0 : c0 + cw, :], in_=RT[:cw, j, :])
    return nc


def _derive_schedule(bsz):
    """bsz[w][s] = padded edge count (cross-core max) for bucket (w, s).

    Buckets are concatenated per shard (s-major, w-minor) with no chunk
    alignment; each shard list is tail-padded to a multiple of 128.  A bucket
    spanning k chunks contributes k matmuls (consecutive, forming one PSUM
    accumulation group).

    Returns (mms, shard_chunklists, ls, bucket_meta)
      mms: per matmul: (shard, chunk_in_shard, w, start, stop, win_first)
      shard_chunklists: per shard, list of sub-gather chunk counts
      ls: per shard, padded index-list length
      bucket_meta: dict (w, s) -> (pos0_in_shard, mm0_index)
    """
    mms = []
    shard_chunklists = []
    ls = []
    bucket_meta = {}
    seen_w = set()
    for s in range(NSHARDS):
        pos = 0
        for w in range(WND):
            size = bsz[w][s]
            if size == 0:
                continue
            wf = w not in seen_w
            seen_w.add(w)
            c0 = pos // P
            c1 = (pos + size - 1) // P
            bucket_meta[(w, s)] = (pos, len(mms))
            for c in range(c0, c1 + 1):
                mms.append((s, c, w, c == c0, c == c1, wf))
            pos += size
        lpad = -(-pos // P) * P
        ls.append(max(lpad, P))
        nch = ls[-1] // P
        lst = []
        while nch > 0:
            take = min(SUBCH, nch)
            lst.append(take)
            nch -= take
        shard_chunklists.append(lst)
    return mms, shard_chunklists, ls, bucket_meta


def _build_agg(bsz):
    from concourse import bacc, mybir
    from concourse.tile import TileContext
    from concourse.library_config import mlp
    from contextlib import ExitStack

    F32 = mybir.dt.float32
    F16 = mybir.dt.float16
    F8 = mybir.dt.float8e4
    I16 = mybir.dt.int16

    mms, shard_chunklists, ls, _bm = _derive_schedule(bsz)
    nmm = len(mms)

    nc = bacc.Bacc("TRN2", target_bir_lowering=False, num_swdge_queues=4)
    tw = nc.dram_tensor("tw", [N, ROWE], F16, kind="ExternalInput")
    idx_d = [
        nc.dram_tensor(f"idx{s}", [P, ls[s] // 16], I16, kind="ExternalInput")
        for s in range(NSHARDS)
    ]
    ohb = nc.dram_tensor("ohb", [P, nmm, P], F8, kind="ExternalInput")
    out = nc.dram_tensor("out", [BN, D_OUT], F32, kind="ExternalOutput")

    with TileContext(nc) as tc, ExitStack() as ctx:
        singles = ctx.enter_context(tc.tile_pool(name="singles", bufs=1))
        gpool = ctx.enter_context(tc.tile_pool(name="gpool", bufs=14))
        opool = ctx.enter_context(tc.tile_pool(name="opool", bufs=4))
        vpool = ctx.enter_context(tc.tile_pool(name="vpool", bufs=4))
        psum = ctx.enter_context(tc.tile_pool(name="psum", bufs=6, space="PSUM"))

        nc.gpsimd.load_library(mlp)

        # window accumulators: [128 nodes, WND, 65] f32 in SBUF
        acc = singles.tile([P, WND, PAY], F32)

        ipool = ctx.enter_context(tc.tile_pool(name="ipool", bufs=20))

        # issue all gathers up front; tile deps throttle via gpool buffers.
        # idx slices are loaded just-in-time per gather so the first gather
        # does not wait on a whole shard's index load.
        gsrc = {}  # (shard, chunk_in_shard) -> (G tile, slot)
        qn = 0
        for s in range(NSHARDS):
            pos = 0
            cbase = 0
            for nch in shard_chunklists[s]:
                nidx = nch * P
                it = ipool.tile([P, SUBCH * 8], I16)
                nc.sync.dma_start(
                    out=it[:, : nidx // 16],
                    in_=idx_d[s][:, pos // 16 : (pos + nidx) // 16],
                )
                G = gpool.tile([P, SUBCH, PAY], F16)
                _raw_dma_gather(
                    nc.gpsimd,
                    G[:, :nch, :],
                    tw[s * SHARD :, :],
                    it[:, : nidx // 16],
                    nidx,
                    PAY,
                    ROWE,
                    queue_num=qn % 4,
                )
                qn += 1
                for j in range(nch):
                    gsrc[(s, cbase + j)] = (G, j)
                pos += nidx
                cbase += nch

        # last contributing shard per window (output emitted right after it)
        s_last = {}
        for w in range(WND):
            for s in range(NSHARDS):
                if bsz[w][s] > 0:
                    s_last[w] = s

        def emit_out(w):
            pw = min(P, BN - w * P)
            r = vpool.tile([P, 1], F32)
            nc.vector.reciprocal(out=r[:pw], in_=acc[:pw, w, D_OUT : D_OUT + 1])
            ot = vpool.tile([P, D_OUT], F32)
            rb = r[:pw].to_broadcast([pw, D_OUT])
            nc.vector.tensor_tensor(
                out=ot[:pw], in0=acc[:pw, w, 0:D_OUT], in1=rb,
                op=mybir.AluOpType.mult,
            )
            nc.sync.dma_start(out=out[w * P : w * P + pw, :], in_=ot[:pw])

        pt_cur = None
        OH = None
        for m in range(nmm):
            s, c, w, bfirst, blast, wfirst = mms[m]
            if m % OHBLK == 0:
                OH = opool.tile([P, OHBLK, P], F8)
                nb = min(OHBLK, nmm - m)
                nc.sync.dma_start(out=OH[:, :nb, :], in_=ohb[:, m : m + nb, :])
            if bfirst:
                pt_cur = psum.tile([P, 512], F32, space="PSUM")
            G, j = gsrc[(s, c)]
            nc.tensor.matmul(
                pt_cur[:, 0:PAY],
                OH[:, m % OHBLK, :],
                G[:, j, :],
                start=bfirst,
                stop=blast,
                skip_group_check=True,
            )
            if blast:
                if wfirst:
                    nc.vector.tensor_copy(out=acc[:, w, :], in_=pt_cur[:, 0:PAY])
                else:
                    nc.vector.tensor_tensor(
                        out=acc[:, w, :], in0=acc[:, w, :], in1=pt_cur[:, 0:PAY],
                        op=mybir.AluOpType.add,
                    )
                if s == s_last[w]:
                    emit_out(w)
    return nc


def _get_nc(key, builder):
    if key not in _CACHE:
        nc = builder()
        nc.finalize()
        _CACHE[key] = nc
    return _CACHE[key]


def _wrap_idx(lst):
    """[n] int16 -> [128, n/16]: wrapped in 16 partitions, replicated x8."""
    n = len(lst)
    t = np.asarray(lst, dtype=np.int16).reshape(n // 16, 16).T
    return np.ascontiguousarray(np.tile(t, (8, 1)))


def kernel(
    padded_neighs,
    mask,
    dst_idx,
    embed_table,
    W_proj,
    a_src,
    a_dst,
):
    import ml_dtypes
    from concourse.bass_utils import run_bass_kernel_spmd

    del dst_idx, a_dst  # constant along softmax axis -> cancels exactly

    trace = bool(int(os.environ.get("KERNEL_TRACE", "0")))
    LAST_PERF.clear()

    padded_neighs = np.asarray(padded_neighs, dtype=np.int32)
    mask = np.asarray(mask, dtype=bool)
    embed_table = np.asarray(embed_table, dtype=np.float32)
    W_proj = np.asarray(W_proj, dtype=np.float32)
    a_src = np.asarray(a_src, dtype=np.float32)

    # compact unmasked neighbors to the front of each row (masked neighbors
    # have exactly zero softmax weight)
    order = np.argsort(~mask, axis=1, kind="stable")
    neigh = np.take_along_axis(padded_neighs, order, axis=1)
    counts = mask.sum(axis=1).astype(np.int64)

    core_ids = list(range(NCORES))

    # ---- launch 1: projection + row weighting (table rows sharded) -----------
    tT = np.ascontiguousarray(embed_table.T.astype(np.float16))
    wT = np.ascontiguousarray(W_proj.T)
    vvec = wT @ a_src  # [256] = W_proj.T @ a_src
    wv = np.empty((P, 2, PAY), dtype=np.float16)
    wv[:, :, :D_OUT] = wT.reshape(2, P, D_OUT).transpose(1, 0, 2)
    wv[:, :, D_OUT] = vvec.reshape(2, P).T

    nc1 = _get_nc("proj", _build_proj)
    in1 = [
        {"tT": np.ascontiguousarray(tT[:, c * R : (c + 1) * R]), "wv": wv}
        for c in core_ids
    ]
    res1 = run_bass_kernel_spmd(nc1, in1, core_ids=core_ids, trace=trace)
    tw = np.concatenate([r["tw"] for r in res1.results], axis=0)  # [N, 128] f16
    global LAST_TW
    LAST_TW = tw

    # ---- host: edge bucketing by (window, shard) ------------------------------
    colmask = np.arange(L)[None, :] < counts[:, None]      # [B, L]
    per_core = []
    sizes = np.zeros((NCORES, WND, NSHARDS), dtype=np.int64)
    for c in core_ids:
        b0 = c * BN
        cm = colmask[b0 : b0 + BN]
        idx_arr = neigh[b0 : b0 + BN][cm].astype(np.int64)   # row-major: b-major
        b_arr = np.repeat(np.arange(BN, dtype=np.int64), counts[b0 : b0 + BN])
        w_arr = b_arr >> 7
        s_arr = idx_arr >> 15
        np.add.at(sizes[c], (w_arr, s_arr), 1)
        per_core.append((idx_arr, b_arr, w_arr, s_arr))

    bsz_arr = sizes.max(axis=0)                              # [WND, NSHARDS]
    bsz = tuple(tuple(int(x) for x in row) for row in bsz_arr)

    mms, shard_chunklists, ls, bucket_meta = _derive_schedule(bsz)
    nmm = len(mms)

    # per-bucket position base and first-mm index as arrays
    pos0 = np.zeros((WND, NSHARDS), dtype=np.int64)
    mm0 = np.zeros((WND, NSHARDS), dtype=np.int64)
    for (w, s), (p0, m0) in bucket_meta.items():
        pos0[w, s] = p0
        mm0[w, s] = m0

    one8 = np.float32(1.0).astype(ml_dtypes.float8_e4m3fn)

    nc2 = _get_nc(("agg", bsz), lambda: _build_agg(bsz))
    in2 = []
    for c in core_ids:
        idx_arr, b_arr, w_arr, s_arr = per_core[c]
        loc_arr = (idx_arr & (SHARD - 1)).astype(np.int16)
        r_arr = (b_arr & 127).astype(np.int64)
        # sort edges by (shard, window, node)
        perm = np.lexsort((b_arr, w_arr, s_arr))
        sw = s_arr[perm]
        ww = w_arr[perm]
        ll = loc_arr[perm]
        rr = r_arr[perm]
        # rank within bucket
        ne = len(sw)
        bucket_id = sw * WND + ww
        change = np.empty(ne, dtype=bool)
        change[0] = True
        change[1:] = bucket_id[1:] != bucket_id[:-1]
        starts = np.flatnonzero(change)
        rank = np.arange(ne, dtype=np.int64) - np.repeat(
            starts, np.diff(np.append(starts, ne))
        )
        pos_in_shard = pos0[ww, sw] + rank
        # matmul id: bucket's first mm + (chunk - bucket's first chunk)
        mm_id = mm0[ww, sw] + (pos_in_shard // P - pos0[ww, sw] // P)
        part = pos_in_shard & 127

        idx_lists = []
        for s in range(NSHARDS):
            lst = np.zeros(ls[s], dtype=np.int16)
            m = sw == s
            lst[pos_in_shard[m]] = ll[m]
            idx_lists.append(_wrap_idx(lst))

        oh = np.zeros((P, nmm, P), dtype=ml_dtypes.float8_e4m3fn)
        oh[part, mm_id, rr] = one8
        d = {"tw": tw, "ohb": oh}
        for s in range(NSHARDS):
            d[f"idx{s}"] = idx_lists[s]
        in2.append(d)

    res2 = run_bass_kernel_spmd(nc2, in2, core_ids=core_ids, trace=trace)
    out = np.concatenate([r["out"] for r in res2.results], axis=0)  # [B, 64]

    if trace:
        LAST_PERF.extend([res1, res2])
    return np.ascontiguousarray(out, dtype=np.float32)


# revision 16
# speedup vs baseline: 1.1884x; 1.1884x over previous
"""Trainium2 Bass kernel for nn_AttnAggregator (GAT-style neighbor aggregation).

Reference computation:
    ep = embed_table @ W_proj.T                      # [N, 64]
    neigh = ep[padded_neighs]                        # [B, L, 64]
    scores = leaky_relu(ep[dst]@a_dst) + leaky_relu(neigh@a_src)
    attn = masked_softmax(scores, axis=L)
    out = sum_l attn * neigh                         # [B, 64]

Algebraic facts used:
  * The dst term is constant along the softmax axis L, so it cancels — the
    output does not depend on dst_idx / a_dst.
  * Masked neighbors have exactly zero softmax weight (exp(-1e9) underflows),
    so only unmasked edges are processed.
  * es(n) = exp(leaky_relu(ep[n]@a_src)) is a pure per-table-row quantity, and
    score(n) = ep[n]@a_src = embed[n]@(W_proj.T@a_src) = embed[n]@v.  So each
    table row can be pre-weighted: row(n) = [es(n)*ep(n,:), es(n)] and
        out[b] = (sum_l row(idx[b,l])[0:64]) / (sum_l row(idx[b,l])[64]).

Strategy (8 NeuronCores):
  Launch 1 (projection, table-row-sharded): each core projects N/8 = 25000
    rows in fp16: one PE matmul per 128-row chunk computes [ep | s] with a
    fused rhs [W^T | v], ACT exp(leaky) + per-partition scale produce fp16
    rows [es*ep | es | pad] written with 256B stride to HBM.
  Launch 2 (aggregation, batch-sharded): each core aggregates B/8 = 6250
    nodes.  Edges are bucketed host-side by (128-node window, 32768-row table
    shard); buckets are concatenated per shard and gathered by
    InstDMAGatherAnt ("dma_gather") SWDGE instructions with int16 shard-local
    indices, 1024 rows per instruction, rotated across SWDGE queues 0-3 so
    all four Q7 pairs generate descriptors concurrently (~3x over one queue).
    Gathered rows land packed [128, chunk, 65] fp16; host-built fp8 one-hot
    matrices map chunk positions to window nodes, so one PE matmul per
    (bucket, chunk) overlap (onehot^T @ rows, PSUM-accumulated per bucket)
    performs the entire segment-sum; bucket partials are DVE-added into
    per-window SBUF accumulators [sum es*ep | sum es], and a final DVE pass
    divides and streams out.  Each bucket's matmuls are consecutive PE
    instructions — interleaved PSUM accumulation groups are broken on HW.
  (dma_gather elem_size of 65 fp16 = 130B is built via a raw InstDMAGatherAnt
   — the bass-level %256 assert is a transpose-mode-only restriction;
   verified on hardware.)
"""

import os
import sys

sys.path.insert(0, "/opt/trn_rl_repo")

import numpy as np

# ---- hardcoded problem shapes -------------------------------------------------
B, L, N = 50000, 50, 200000
D_IN, D_OUT = 256, 64
NCORES = 8
R = N // NCORES        # 25000 table rows per core (launch 1)
BN = B // NCORES       # 6250 batch nodes per core (launch 2)
P = 128
ROWE = 128             # fp16 elements per padded table row (256B stride)
PAY = D_OUT + 1        # 65 gathered elements: [64 weighted feats | es]
SHARD = 32768          # dma_gather int16 index range per table slice
NSHARDS = (N + SHARD - 1) // SHARD   # 7
WND = (BN + P - 1) // P              # 49 node windows per core
SUBCH = 8              # 128-row chunks per dma_gather (1024 idx; 2048 hangs)
OHBLK = 64             # one-hot matrices per DMA block

_CACHE = {}
LAST_PERF = []         # filled when KERNEL_TRACE=1: list of BassKernelResults
LAST_TW = None         # debug: the fp16 weighted table from launch 1


def _raw_dma_gather(g, out_ap, in_ap, idxs_ap, num_idxs, elem_size, elem_step,
                    queue_num=0):
    """dma_gather with arbitrary elem_size (the bass %256 assert is a
    transpose-mode restriction; non-transpose handles any element bytes)."""
    from concourse import mybir
    from concourse._compat import exact_div

    stride_bytes = elem_step * mybir.dt.size(in_ap.dtype)
    return g.add_instruction(
        mybir.InstDMAGatherAnt(
            name=g.bass.get_next_instruction_name(),
            ins=[
                *g.lower_ap_dma(in_ap, for_custom_bir_dma=True),
                g.lower_ap(idxs_ap),
                g.lower_val_access(g.to_reg(num_idxs)),
            ],
            outs=[g.lower_ap(out_ap)],
            transpose=False,
            num_idxs=num_idxs,
            elem_size=elem_size,
            stride_bytes_256=exact_div(stride_bytes, 256),
            gen_mode=0,
            single_packet=True,
            queue_num=queue_num,
            sbuf_tokens_per_rank=0,
            sbuf_free_dim_per_rank=0,
            sbuf_free_dim_pad_per_rank=0,
            sbuf_byte_offset=0,
        )
    )


def _build_proj():
    """Launch 1: tw[r, :] = [es*ep (64) | es | garbage pad] fp16, 256B stride."""
    from concourse import bacc, mybir
    from concourse.tile import TileContext
    from contextlib import ExitStack

    F32 = mybir.dt.float32
    F16 = mybir.dt.float16
    nc = bacc.Bacc("TRN2", target_bir_lowering=False)
    tT = nc.dram_tensor("tT", [D_IN, R], F16, kind="ExternalInput")
    wv = nc.dram_tensor("wv", [P, 2, PAY], F16, kind="ExternalInput")
    tw = nc.dram_tensor("tw", [R, ROWE], F16, kind="ExternalOutput")

    CPB = 7
    BLK = P * CPB

    with TileContext(nc) as tc, ExitStack() as ctx:
        singles = ctx.enter_context(tc.tile_pool(name="singles", bufs=1))
        tpool = ctx.enter_context(tc.tile_pool(name="tpool", bufs=4))
        spool = ctx.enter_context(tc.tile_pool(name="spool", bufs=6))
        rpool = ctx.enter_context(tc.tile_pool(name="rpool", bufs=4))
        psum = ctx.enter_context(tc.tile_pool(name="psum", bufs=6, space="PSUM"))

        wv_sb = singles.tile([P, 2, PAY], F16)
        nc.sync.dma_start(out=wv_sb[:], in_=wv[:, :, :])
        tTr = tT.rearrange("(k p) r -> p k r", p=P)

        for B0 in range(0, R, BLK):
            wcols = min(BLK, R - B0)
            nj = (wcols + P - 1) // P
            tt = tpool.tile([P, 2, BLK], F16)
            nc.sync.dma_start(out=tt[:, :, :wcols], in_=tTr[:, :, B0 : B0 + wcols])
            pse = psum.tile([P, CPB, PAY], F32, space="PSUM")
            for j in range(nj):
                c0 = B0 + j * P
                cw = min(P, R - c0)
                nc.tensor.matmul(
                    pse[:cw, j, :], tt[:, 0, j * P : j * P + cw], wv_sb[:, 0, :],
                    start=True, stop=False,
                )
                nc.tensor.matmul(
                    pse[:cw, j, :], tt[:, 1, j * P : j * P + cw], wv_sb[:, 1, :],
                    start=False, stop=True,
                )
            sc = spool.tile([P, CPB], F32)
            nc.vector.tensor_copy(out=sc[:, :nj], in_=pse[:, :nj, D_OUT])
            lr = spool.tile([P, CPB], F32)
            nc.vector.scalar_tensor_tensor(
                out=lr[:, :nj],
                in0=sc[:, :nj],
                scalar=0.2,
                in1=sc[:, :nj],
                op0=mybir.AluOpType.mult,
                op1=mybir.AluOpType.max,
            )
            es = spool.tile([P, CPB], F32)
            nc.scalar.activation(
                out=es[:, :nj], in_=lr[:, :nj],
                func=mybir.ActivationFunctionType.Exp,
            )
            RT = rpool.tile([P, CPB, ROWE], F16)
            esb = es[:, :nj].to_broadcast([P, nj, D_OUT])
            nc.vector.tensor_tensor(
                out=RT[:, :nj, 0:D_OUT], in0=pse[:, :nj, 0:D_OUT], in1=esb,
                op=mybir.AluOpType.mult,
            )
            nc.vector.tensor_copy(out=RT[:, :nj, D_OUT], in_=es[:, :nj])
            if wcols == BLK:
                nc.sync.dma_start(
                    out=tw[B0 : B0 + BLK, :].rearrange("(j p) e -> p j e", p=P),
                    in_=RT[:, :, :],
                )
            else:
                for j in range(nj):
                    c0 = B0 + j * P
                    cw = min(P, R - c0)
                    nc.sync.dma_start(out=tw[c0 : c0 + cw, :], in_=RT[:cw, j, :])
    return nc


def _derive_schedule(bsz):
    """bsz[w][s] = padded edge count (cross-core max) for bucket (w, s).

    Buckets are concatenated per shard (s-major, w-minor) with no chunk
    alignment; each shard list is tail-padded to a multiple of 128.  A bucket
    spanning k chunks contributes k matmuls (consecutive, forming one PSUM
    accumulation group).

    Returns (mms, shard_chunklists, ls, bucket_meta)
      mms: per matmul: (shard, chunk_in_shard, w, start, stop, win_first)
      shard_chunklists: per shard, list of sub-gather chunk counts
      ls: per shard, padded index-list length
      bucket_meta: dict (w, s) -> (pos0_in_shard, mm0_index)
    """
    mms = []
    shard_chunklists = []
    ls = []
    bucket_meta = {}
    seen_w = set()
    for s in range(NSHARDS):
        pos = 0
        for w in range(WND):
            size = bsz[w][s]
            if size == 0:
                continue
            wf = w not in seen_w
            seen_w.add(w)
            c0 = pos // P
            c1 = (pos + size - 1) // P
            bucket_meta[(w, s)] = (pos, len(mms))
            for c in range(c0, c1 + 1):
                mms.append((s, c, w, c == c0, c == c1, wf))
            pos += size
        lpad = -(-pos // P) * P
        ls.append(max(lpad, P))
        nch = ls[-1] // P
        lst = []
        while nch > 0:
            take = min(SUBCH, nch)
            lst.append(take)
            nch -= take
        shard_chunklists.append(lst)
    return mms, shard_chunklists, ls, bucket_meta


def _build_agg(bsz):
    from concourse import bacc, mybir
    from concourse.tile import TileContext
    from concourse.library_config import mlp
    from contextlib import ExitStack

    F32 = mybir.dt.float32
    F16 = mybir.dt.float16
    F8 = mybir.dt.float8e4
    I16 = mybir.dt.int16

    mms, shard_chunklists, ls, _bm = _derive_schedule(bsz)
    nmm = len(mms)

    nc = bacc.Bacc("TRN2", target_bir_lowering=False, num_swdge_queues=4)
    tw = nc.dram_tensor("tw", [N, ROWE], F16, kind="ExternalInput")
    idx_d = [
        nc.dram_tensor(f"idx{s}", [P, ls[s] // 16], I16, kind="ExternalInput")
        for s in range(NSHARDS)
    ]
    ohb = nc.dram_tensor("ohb", [P, nmm, P], F8, kind="ExternalInput")
    out = nc.dram_tensor("out", [BN, D_OUT], F32, kind="ExternalOutput")

    with TileContext(nc) as tc, ExitStack() as ctx:
        singles = ctx.enter_context(tc.tile_pool(name="singles", bufs=1))
        gpool = ctx.enter_context(tc.tile_pool(name="gpool", bufs=14))
        opool = ctx.enter_context(tc.tile_pool(name="opool", bufs=4))
        vpool = ctx.enter_context(tc.tile_pool(name="vpool", bufs=4))
        psum = ctx.enter_context(tc.tile_pool(name="psum", bufs=6, space="PSUM"))

        nc.gpsimd.load_library(mlp)

        # window accumulators: [128 nodes, WND, 65] f32 in SBUF
        acc = singles.tile([P, WND, PAY], F32)

        its = []
        for s in range(NSHARDS):
            it = singles.tile([P, ls[s] // 16], I16)
            nc.sync.dma_start(out=it[:], in_=idx_d[s][:, :])
            its.append(it)

        # issue all gathers up front; tile deps throttle via gpool buffers
        gsrc = {}  # (shard, chunk_in_shard) -> (G tile, slot)
        qn = 0
        for s in range(NSHARDS):
            pos = 0
            cbase = 0
            for nch in shard_chunklists[s]:
                nidx = nch * P
                G = gpool.tile([P, SUBCH, PAY], F16)
                _raw_dma_gather(
                    nc.gpsimd,
                    G[:, :nch, :],
                    tw[s * SHARD :, :],
                    its[s][:, pos // 16 : (pos + nidx) // 16],
                    nidx,
                    PAY,
                    ROWE,
                    queue_num=qn % 4,
                )
                qn += 1
                for j in range(nch):
                    gsrc[(s, cbase + j)] = (G, j)
                pos += nidx
                cbase += nch

        # last contributing shard per window (output emitted right after it)
        s_last = {}
        for w in range(WND):
            for s in range(NSHARDS):
                if bsz[w][s] > 0:
                    s_last[w] = s

        def emit_out(w):
            pw = min(P, BN - w * P)
            r = vpool.tile([P, 1], F32)
            nc.vector.reciprocal(out=r[:pw], in_=acc[:pw, w, D_OUT : D_OUT + 1])
            ot = vpool.tile([P, D_OUT], F32)
            rb = r[:pw].to_broadcast([pw, D_OUT])
            nc.vector.tensor_tensor(
                out=ot[:pw], in0=acc[:pw, w, 0:D_OUT], in1=rb,
                op=mybir.AluOpType.mult,
            )
            nc.sync.dma_start(out=out[w * P : w * P + pw, :], in_=ot[:pw])

        pt_cur = None
        OH = None
        for m in range(nmm):
            s, c, w, bfirst, blast, wfirst = mms[m]
            if m % OHBLK == 0:
                OH = opool.tile([P, OHBLK, P], F8)
                nb = min(OHBLK, nmm - m)
                nc.sync.dma_start(out=OH[:, :nb, :], in_=ohb[:, m : m + nb, :])
            if bfirst:
                pt_cur = psum.tile([P, 512], F32, space="PSUM")
            G, j = gsrc[(s, c)]
            nc.tensor.matmul(
                pt_cur[:, 0:PAY],
                OH[:, m % OHBLK, :],
                G[:, j, :],
                start=bfirst,
                stop=blast,
                skip_group_check=True,
            )
            if blast:
                if wfirst:
                    nc.vector.tensor_copy(out=acc[:, w, :], in_=pt_cur[:, 0:PAY])
                else:
                    nc.vector.tensor_tensor(
                        out=acc[:, w, :], in0=acc[:, w, :], in1=pt_cur[:, 0:PAY],
                        op=mybir.AluOpType.add,
                    )
                if s == s_last[w]:
                    emit_out(w)
    return nc


def _get_nc(key, builder):
    if key not in _CACHE:
        nc = builder()
        nc.finalize()
        _CACHE[key] = nc
    return _CACHE[key]


def _wrap_idx(lst):
    """[n] int16 -> [128, n/16]: wrapped in 16 partitions, replicated x8."""
    n = len(lst)
    t = np.asarray(lst, dtype=np.int16).reshape(n // 16, 16).T
    return np.ascontiguousarray(np.tile(t, (8, 1)))


def kernel(
    padded_neighs,
    mask,
    dst_idx,
    embed_table,
    W_proj,
    a_src,
    a_dst,
):
    import ml_dtypes
    from concourse.bass_utils import run_bass_kernel_spmd

    del dst_idx, a_dst  # constant along softmax axis -> cancels exactly

    trace = bool(int(os.environ.get("KERNEL_TRACE", "0")))
    LAST_PERF.clear()

    padded_neighs = np.asarray(padded_neighs, dtype=np.int32)
    mask = np.asarray(mask, dtype=bool)
    embed_table = np.asarray(embed_table, dtype=np.float32)
    W_proj = np.asarray(W_proj, dtype=np.float32)
    a_src = np.asarray(a_src, dtype=np.float32)

    # compact unmasked neighbors to the front of each row (masked neighbors
    # have exactly zero softmax weight)
    order = np.argsort(~mask, axis=1, kind="stable")
    neigh = np.take_along_axis(padded_neighs, order, axis=1)
    counts = mask.sum(axis=1).astype(np.int64)

    core_ids = list(range(NCORES))

    # ---- launch 1: projection + row weighting (table rows sharded) -----------
    tT = np.ascontiguousarray(embed_table.T.astype(np.float16))
    wT = np.ascontiguousarray(W_proj.T)
    vvec = wT @ a_src  # [256] = W_proj.T @ a_src
    wv = np.empty((P, 2, PAY), dtype=np.float16)
    wv[:, :, :D_OUT] = wT.reshape(2, P, D_OUT).transpose(1, 0, 2)
    wv[:, :, D_OUT] = vvec.reshape(2, P).T

    nc1 = _get_nc("proj", _build_proj)
    in1 = [
        {"tT": np.ascontiguousarray(tT[:, c * R : (c + 1) * R]), "wv": wv}
        for c in core_ids
    ]
    res1 = run_bass_kernel_spmd(nc1, in1, core_ids=core_ids, trace=trace)
    tw = np.concatenate([r["tw"] for r in res1.results], axis=0)  # [N, 128] f16
    global LAST_TW
    LAST_TW = tw

    # ---- host: edge bucketing by (window, shard) ------------------------------
    colmask = np.arange(L)[None, :] < counts[:, None]      # [B, L]
    per_core = []
    sizes = np.zeros((NCORES, WND, NSHARDS), dtype=np.int64)
    for c in core_ids:
        b0 = c * BN
        cm = colmask[b0 : b0 + BN]
        idx_arr = neigh[b0 : b0 + BN][cm].astype(np.int64)   # row-major: b-major
        b_arr = np.repeat(np.arange(BN, dtype=np.int64), counts[b0 : b0 + BN])
        w_arr = b_arr >> 7
        s_arr = idx_arr >> 15
        np.add.at(sizes[c], (w_arr, s_arr), 1)
        per_core.append((idx_arr, b_arr, w_arr, s_arr))

    bsz_arr = sizes.max(axis=0)                              # [WND, NSHARDS]
    bsz = tuple(tuple(int(x) for x in row) for row in bsz_arr)

    mms, shard_chunklists, ls, bucket_meta = _derive_schedule(bsz)
    nmm = len(mms)

    # per-bucket position base and first-mm index as arrays
    pos0 = np.zeros((WND, NSHARDS), dtype=np.int64)
    mm0 = np.zeros((WND, NSHARDS), dtype=np.int64)
    for (w, s), (p0, m0) in bucket_meta.items():
        pos0[w, s] = p0
        mm0[w, s] = m0

    one8 = np.float32(1.0).astype(ml_dtypes.float8_e4m3fn)

    nc2 = _get_nc(("agg", bsz), lambda: _build_agg(bsz))
    in2 = []
    for c in core_ids:
        idx_arr, b_arr, w_arr, s_arr = per_core[c]
        loc_arr = (idx_arr & (SHARD - 1)).astype(np.int16)
        r_arr = (b_arr & 127).astype(np.int64)
        # sort edges by (shard, window, node)
        perm = np.lexsort((b_arr, w_arr, s_arr))
        sw = s_arr[perm]
        ww = w_arr[perm]
        ll = loc_arr[perm]
        rr = r_arr[perm]
        # rank within bucket
        ne = len(sw)
        bucket_id = sw * WND + ww
        change = np.empty(ne, dtype=bool)
        change[0] = True
        change[1:] = bucket_id[1:] != bucket_id[:-1]
        starts = np.flatnonzero(change)
        rank = np.arange(ne, dtype=np.int64) - np.repeat(
            starts, np.diff(np.append(starts, ne))
        )
        pos_in_shard = pos0[ww, sw] + rank
        # matmul id: bucket's first mm + (chunk - bucket's first chunk)
        mm_id = mm0[ww, sw] + (pos_in_shard // P - pos0[ww, sw] // P)
        part = pos_in_shard & 127

        idx_lists = []
        for s in range(NSHARDS):
            lst = np.zeros(ls[s], dtype=np.int16)
            m = sw == s
            lst[pos_in_shard[m]] = ll[m]
            idx_lists.append(_wrap_idx(lst))

        oh = np.zeros((P, nmm, P), dtype=ml_dtypes.float8_e4m3fn)
        oh[part, mm_id, rr] = one8
        d = {"tw": tw, "ohb": oh}
        for s in range(NSHARDS):
            d[f"idx{s}"] = idx_lists[s]
        in2.append(d)

    res2 = run_bass_kernel_spmd(nc2, in2, core_ids=core_ids, trace=trace)
    out = np.concatenate([r["out"] for r in res2.results], axis=0)  # [B, 64]

    if trace:
        LAST_PERF.extend([res1, res2])
    return np.ascontiguousarray(out, dtype=np.float32)
